# revision 26
# baseline (speedup 1.0000x reference)
"""Trainium2 Bass kernel for BaseCausalWanSelfAttention (local+sink sparse attention
with interleaved rotary), SPMD across 8 NeuronCores.

Sharding: the 24 (batch, head) pairs are split 3-per-core across 8 cores; each
core runs full local+sink attention for its pairs independently (no collectives).

Design notes (per 128-key x query-span score tile, transposed layout [k, q]):
  - QK matmuls in bf16 (1 cycle/row).
  - probs stored as fp8e4; PV and denominator matmuls run fp8 DoubleRow pairs
    (two adjacent k-tiles contracted per pass) where possible.
  - exp is split: full tiles on the ACT engine (Exp -> fp8 out, batched over
    up to 1536 psum columns); mostly-masked tiles on DVE via a Schraudolph
    bit-trick (one scalar_tensor_tensor: i8 = (score + c0) * pattern, where
    pattern = SCALE*8/ln2 on allowed positions and 0 on masked ones, and the
    int8 bits reinterpret as fp8e4 = approx exp). Masking is free there.
  - rotary uses host-provided row-swapped copies of q/k; the swap-multiply
    runs on GpSimd (Pool), the rest on DVE, all bf16.
  - output transpose via DMA crossbar (dma_start_transpose), not the PE.
"""
import sys

sys.path.insert(0, "/opt/trn_rl_repo")

import numpy as np
import ml_dtypes

import concourse.bacc as bacc
import concourse.tile as tile
import concourse.mybir as mybir

dt = mybir.dt
BF16 = ml_dtypes.bfloat16
FP8 = ml_dtypes.float8_e4m3

# Problem config (hardcoded per contest contract)
B, S, H, D = 2, 3072, 12, 128
LOCAL_WINDOW = 1560
SINK = 128
N_CORES = 8
PER_CORE = (B * H) // N_CORES  # 3
QB = 512
NQC = QB // 128
SCALE = 1.0 / float(np.sqrt(D))

# Schraudolph exp-to-fp8e4 constants: i8 = round((s + C0) * M0) bit-read as
# e4m3 approximates exp(s * SCALE).  M0 = SCALE*8/ln2; C0 = (56 - C)/M0 with
# C=0.45 tuned numerically (rms rel err ~3.1%, bias ~5e-4).
M0 = SCALE * 8.0 / np.log(2.0)
C0 = (56.0 - 0.45) / M0

GROUP_COLS = 1536  # psum columns per score group (3 banks)


def _window_partial_deltas(w):
    out = {}
    for d in range((w - 127 + 127) // 128, (w + 127) // 128 + 1):
        t = w - 128 * d
        if -127 <= t <= 127:
            out[d] = t
    return out


PARTIAL = _window_partial_deltas(LOCAL_WINDOW)  # {12: 24, 13: -104}
W_DELTAS = sorted(PARTIAL)  # [12, 13]
MAX_DELTA = max(PARTIAL)


def chunk_kinds(qb, kj):
    """Valid chunks of k-tile kj for query block qb: list of (t, kind),
    kind in {"full", "diag", ("win", delta)}."""
    kinds = []
    for t in range(NQC):
        qi = NQC * qb + t
        if kj == 0:
            kinds.append((t, "diag" if qi == 0 else "full"))
            continue
        delta = qi - kj
        if delta < 0 or delta > MAX_DELTA:
            continue
        if delta == 0:
            kinds.append((t, "diag"))
        elif delta in PARTIAL:
            kinds.append((t, ("win", delta)))
        else:
            kinds.append((t, "full"))
    return kinds


def kj_list(qb):
    n_ktiles = S // 128
    hi = min(NQC * qb + NQC - 1, n_ktiles - 1)
    lo = max(1, NQC * qb - MAX_DELTA)
    return [0] + list(range(lo, hi + 1))


def plan_qb(qb):
    """Plan tiles + groups for a query block.

    Tile: dict(kj, t0, span, exp in {"act","stt"}, pat, post)
      pat  (stt): ("dpat"|"fw", lo, hi) slice of the pattern const
      post (act): (c_lo, c_hi, maskname) columns within the tile to zero after
    Group: dict(tiles=[...], offs=[...], act_cols=n, pairs=[i...])
      pair indices are positions of DoubleRow pair starts in tiles[].

    qb 0 runs fully in bf16 (ACT exp + post masks): its softmax sees few keys,
    so fp8 quantization error would pass straight into the output there.
    """
    tiles = []
    for kj in kj_list(qb):
        kinds = chunk_kinds(qb, kj)
        assert kinds
        t0, t1 = kinds[0][0], kinds[-1][0] + 1
        span = (t1 - t0) * 128
        masked = [(t, k) for t, k in kinds if k != "full"]
        tl = dict(kj=kj, t0=t0, span=span, post=None, pat=None)
        if not masked:
            tl["exp"] = "act"
        elif qb == 0:
            tl["exp"] = "act"
            assert masked[0][1] == "diag" and masked[0][0] == t0, (qb, kj)
            tl["post"] = (0, 128, "mD")
        elif span <= 384:
            tl["exp"] = "stt"
            if kinds[0][1] == "diag":
                # diag-first pattern [D, F, F, ...]
                assert all(k == "full" for _, k in kinds[1:])
                tl["pat"] = ("dpat", 0, span)
            else:
                # window-last pattern: suffix of [F, F, W12, W13]
                tl["pat"] = ("fw", 512 - span, 512)
        else:
            tl["exp"] = "act"
            # masked chunks are a contiguous prefix (diag) or suffix (win)
            mts = [t - t0 for t, _ in masked]
            kindnames = [k if isinstance(k, str) else f"w{k[1]}" for _, k in masked]
            if masked[0][1] == "diag":
                assert mts == [0]
                tl["post"] = (0, 128, "mD")
            else:
                c_lo = mts[0] * 128
                assert mts == list(range(mts[0], mts[0] + len(mts)))
                if kindnames == ["w12", "w13"]:
                    tl["post"] = (c_lo, c_lo + 256, "mWp")
                elif kindnames == ["w12"]:
                    tl["post"] = (c_lo, c_lo + 128, "mW12")
                elif kindnames == ["w13"]:
                    tl["post"] = (c_lo, c_lo + 128, "mW13")
                else:
                    raise AssertionError((qb, kj, kindnames))
        tiles.append(tl)

    # Pair adjacent-kj ACT tiles with identical (t0, span) for DoubleRow.
    act_idx = [i for i, t in enumerate(tiles) if t["exp"] == "act"]
    paired = {}
    i = 0
    while i + 1 < len(act_idx):
        a, b = act_idx[i], act_idx[i + 1]
        ta, tb = tiles[a], tiles[b]
        if (
            tb["kj"] == ta["kj"] + 1
            and ta["t0"] == tb["t0"]
            and ta["span"] == tb["span"]
        ):
            paired[a] = b
            i += 2
        else:
            i += 1

    # Pack into groups of <= GROUP_COLS/512 psum bank slots.  Every tile gets
    # a 512-aligned psum slot (a matmul output must not cross a 2KB psum bank
    # boundary); a pair takes two adjacent slots.
    units = []
    used = set()
    for a, b in paired.items():
        units.append([a, b])
        used.update((a, b))
    for i, t in enumerate(tiles):
        if i not in used:
            units.append([i])
    n_slots = GROUP_COLS // 512
    groups = []
    cur, slots = [], 0
    for u in units:
        if cur and slots + len(u) > n_slots:
            groups.append(cur)
            cur, slots = [], 0
        cur.append(u)
        slots += len(u)
    if cur:
        groups.append(cur)

    out = []
    for g in groups:
        gtiles, offs, pairs = [], [], []
        slot = 0
        # act tiles first within the group so the exp ranges merge
        for u in sorted(g, key=lambda u: 0 if tiles[u[0]]["exp"] == "act" else 1):
            if len(u) == 2:
                pairs.append(len(gtiles))
            for i in u:
                gtiles.append(tiles[i])
                offs.append(slot * 512)
                slot += 1
        # maximal contiguous psum ranges covering the act tiles (for exp)
        act_segs = []
        for tl, off in zip(gtiles, offs):
            if tl["exp"] != "act":
                continue
            if act_segs and act_segs[-1][1] == off:
                act_segs[-1][1] = off + tl["span"]
            else:
                act_segs.append([off, off + tl["span"]])
        out.append(dict(tiles=gtiles, offs=offs, act_segs=act_segs, pairs=pairs))
    return out


def build_nc(per_core=PER_CORE):
    nqb = S // QB
    nc = bacc.Bacc("TRN2", target_bir_lowering=False, debug=False)

    qT = nc.declare_dram_parameter("qT", [per_core, 128, S], dt.bfloat16, isOutput=False)
    kT = nc.declare_dram_parameter("kT", [per_core, 128, S], dt.bfloat16, isOutput=False)
    qTs = nc.declare_dram_parameter("qTs", [per_core, 128, S], dt.bfloat16, isOutput=False)
    kTs = nc.declare_dram_parameter("kTs", [per_core, 128, S], dt.bfloat16, isOutput=False)
    v8 = nc.declare_dram_parameter("v8", [per_core, S, 128], dt.float8e4, isOutput=False)
    vb = nc.declare_dram_parameter("vb", [per_core, 512, 128], dt.bfloat16, isOutput=False)
    cexpT = nc.declare_dram_parameter("cexpT", [128, S], dt.bfloat16, isOutput=False)
    ssigT = nc.declare_dram_parameter("ssigT", [128, S], dt.bfloat16, isOutput=False)
    ones8 = nc.declare_dram_parameter("ones8", [128, 256], dt.float8e4, isOutput=False)
    onesb = nc.declare_dram_parameter("onesb", [128, 128], dt.bfloat16, isOutput=False)
    mDb = nc.declare_dram_parameter("mDb", [128, 128], dt.bfloat16, isOutput=False)
    dpat = nc.declare_dram_parameter("dpat", [128, 512], dt.float32, isOutput=False)
    fw = nc.declare_dram_parameter("fw", [128, 512], dt.float32, isOutput=False)
    mD = nc.declare_dram_parameter("mD", [128, 128], dt.float8e4, isOutput=False)
    mW12 = nc.declare_dram_parameter("mW12", [128, 128], dt.float8e4, isOutput=False)
    mW13 = nc.declare_dram_parameter("mW13", [128, 128], dt.float8e4, isOutput=False)
    mWp = nc.declare_dram_parameter("mWp", [128, 256], dt.float8e4, isOutput=False)
    out = nc.declare_dram_parameter("out", [per_core, S, 128], dt.bfloat16, isOutput=True)

    with tile.TileContext(nc) as tc:
        with (
            tc.tile_pool(name="const", bufs=1) as cpool,
            tc.tile_pool(name="big", bufs=2) as bigpool,
            tc.tile_pool(name="probs", bufs=4) as ppool,
            tc.tile_pool(name="tail", bufs=2) as tpool,
            tc.tile_pool(name="ps_sc", bufs=2, space="PSUM") as ps_sc,
            tc.tile_pool(name="ps_out", bufs=1, space="PSUM") as ps_out,
            tc.tile_pool(name="ps_den", bufs=1, space="PSUM") as ps_den,
        ):
            # constants
            cexp_sb = cpool.tile([128, S], dt.bfloat16, tag="cexp")
            ssig_sb = cpool.tile([128, S], dt.bfloat16, tag="ssig")
            nc.sync.dma_start(out=cexp_sb[:, 0:1024], in_=cexpT[:, 0:1024])
            nc.sync.dma_start(out=ssig_sb[:, 0:1024], in_=ssigT[:, 0:1024])
            ones8_sb = cpool.tile([128, 256], dt.float8e4, tag="ones8")
            onesb_sb = cpool.tile([128, 128], dt.bfloat16, tag="onesb")
            mDb_sb = cpool.tile([128, 128], dt.bfloat16, tag="mDb")
            dpat_sb = cpool.tile([128, 512], dt.float32, tag="dpat")
            fw_sb = cpool.tile([128, 512], dt.float32, tag="fw")
            mask_sb = {
                "mD": cpool.tile([128, 128], dt.float8e4, tag="mD", name="mD"),
                "mW12": cpool.tile([128, 128], dt.float8e4, tag="mW12", name="mW12"),
                "mW13": cpool.tile([128, 128], dt.float8e4, tag="mW13", name="mW13"),
                "mWp": cpool.tile([128, 256], dt.float8e4, tag="mWp", name="mWp"),
            }
            pat_sb = {"dpat": dpat_sb, "fw": fw_sb}

            def load_consts_rest():
                nc.sync.dma_start(out=ones8_sb[:], in_=ones8[:])
                nc.sync.dma_start(out=onesb_sb[:], in_=onesb[:])
                nc.sync.dma_start(out=mDb_sb[:], in_=mDb[:])
                nc.sync.dma_start(out=dpat_sb[:], in_=dpat[:])
                nc.sync.dma_start(out=fw_sb[:], in_=fw[:])
                nc.sync.dma_start(out=mask_sb["mD"][:], in_=mD[:])
                nc.sync.dma_start(out=mask_sb["mW12"][:], in_=mW12[:])
                nc.sync.dma_start(out=mask_sb["mW13"][:], in_=mW13[:])
                nc.sync.dma_start(out=mask_sb["mWp"][:], in_=mWp[:])
                for c2 in range(1, S // 1024):
                    sl2 = slice(c2 * 1024, (c2 + 1) * 1024)
                    nc.sync.dma_start(out=cexp_sb[:, sl2], in_=cexpT[:, sl2])
                    nc.sync.dma_start(out=ssig_sb[:, sl2], in_=ssigT[:, sl2])

            def load(u):
                qraw = bigpool.tile([128, S], dt.bfloat16, tag="qraw")
                kraw = bigpool.tile([128, S], dt.bfloat16, tag="kraw")
                qsw = bigpool.tile([128, S], dt.bfloat16, tag="qsw")
                ksw = bigpool.tile([128, S], dt.bfloat16, tag="ksw")
                v_sb = bigpool.tile([128, S], dt.float8e4, tag="v8")
                nc.sync.dma_start(out=qraw[:], in_=qT[u][:])
                nc.sync.dma_start(out=kraw[:], in_=kT[u][:])
                nc.sync.dma_start(out=qsw[:], in_=qTs[u][:])
                nc.sync.dma_start(out=ksw[:], in_=kTs[u][:])
                nc.sync.dma_start(
                    out=v_sb[:].rearrange("p (n d) -> p n d", d=128),
                    in_=v8[u].rearrange("(n p) d -> p n d", p=128),
                )
                vb_sb = bigpool.tile([128, 512], dt.bfloat16, tag="vb")
                nc.sync.dma_start(
                    out=vb_sb[:].rearrange("p (n d) -> p n d", d=128),
                    in_=vb[u].rearrange("(n p) d -> p n d", p=128),
                )
                rq = bigpool.tile([128, S], dt.bfloat16, tag="rq")
                rk = bigpool.tile([128, S], dt.bfloat16, tag="rk")
                swm = bigpool.tile([128, S], dt.bfloat16, tag="swm")
                return dict(
                    q=qraw, k=kraw, qs=qsw, ks=ksw, v=v_sb, vb=vb_sb,
                    rq=rq, rk=rk, sw=swm,
                )

            def rotary(t, lo, hi, pool_add=False):
                """rq = q*cexp + qsw*ssig on [lo,hi); swap-mult on Pool.
                pool_add also moves the final add to Pool (for the next-unit
                rotary that runs while this unit's attention keeps DVE busy)."""
                step = 1024 if (hi - lo) % 1024 == 0 else 512
                for raw, swr, r in ((t["q"], t["qs"], t["rq"]), (t["k"], t["ks"], t["rk"])):
                    for c in range(lo // step, hi // step):
                        sl = slice(c * step, (c + 1) * step)
                        nc.gpsimd.tensor_mul(t["sw"][:, sl], swr[:, sl], ssig_sb[:, sl])
                        nc.vector.tensor_mul(r[:, sl], raw[:, sl], cexp_sb[:, sl])
                        if pool_add:
                            nc.gpsimd.tensor_add(r[:, sl], r[:, sl], t["sw"][:, sl])
                        else:
                            nc.vector.tensor_add(r[:, sl], r[:, sl], t["sw"][:, sl])

            state = {"pv": [], "tail": None}

            def flush_pv():
                if state["pv"]:
                    state["pv"].pop(0)()

            def flush_all():
                while state["pv"]:
                    flush_pv()

            def attention_qb(u, t, qb):
                groups = plan_qb(qb)
                n_groups = len(groups)
                rq, rk, v_sb = t["rq"], t["rk"], t["v"]
                qbctx = {}

                def get_psums():
                    if "outT" not in qbctx:
                        qbctx["outT"] = ps_out.tile(
                            [128, QB], dt.float32, tag="outT", name="outT"
                        )
                        qbctx["den"] = ps_den.tile(
                            [128, QB], dt.float32, tag="den", name="den"
                        )
                    return qbctx["outT"], qbctx["den"]

                bf = qb == 0  # query-block 0 runs in bf16 (see plan_qb)
                for gi, g in enumerate(groups):
                    gtiles, offs = g["tiles"], g["offs"]
                    if bf:
                        probs = ppool.tile([128, GROUP_COLS], dt.bfloat16, tag="probsb")
                    else:
                        probs = ppool.tile([128, GROUP_COLS], dt.float8e4, tag="probs")
                    sc = ps_sc.tile([128, GROUP_COLS], dt.float32, tag="sc")
                    # QK for every tile in the group
                    for tl, off in zip(gtiles, offs):
                        csl = slice(qb * QB + tl["t0"] * 128, qb * QB + tl["t0"] * 128 + tl["span"])
                        ksl = slice(tl["kj"] * 128, (tl["kj"] + 1) * 128)
                        nc.tensor.matmul(
                            sc[:, off : off + tl["span"]],
                            rk[:, ksl], rq[:, csl], start=True, stop=True,
                        )
                    # exp: one ACT instruction per contiguous act psum range
                    for lo, hi in g["act_segs"]:
                        nc.scalar.activation(
                            probs[:, lo:hi],
                            sc[:, lo:hi],
                            mybir.ActivationFunctionType.Exp,
                            scale=SCALE,
                        )
                    # stt tiles: Schraudolph exp+mask in one DVE op each
                    for tl, off in zip(gtiles, offs):
                        if tl["exp"] == "stt":
                            pname, plo, phi = tl["pat"]
                            nc.vector.scalar_tensor_tensor(
                                out=probs[:, off : off + tl["span"]].bitcast(dt.int8),
                                in0=sc[:, off : off + tl["span"]],
                                scalar=C0,
                                in1=pat_sb[pname][:, plo:phi],
                                op0=mybir.AluOpType.add,
                                op1=mybir.AluOpType.mult,
                            )
                    # post-masks for act tiles with masked chunks
                    for tl, off in zip(gtiles, offs):
                        if tl["post"] is not None:
                            c_lo, c_hi, mn = tl["post"]
                            m = mDb_sb if bf else mask_sb[mn]
                            assert not bf or mn == "mD"
                            nc.vector.tensor_mul(
                                probs[:, off + c_lo : off + c_hi],
                                probs[:, off + c_lo : off + c_hi],
                                m[:],
                            )

                    is_last = gi == n_groups - 1

                    def pv_emit(
                        u=u, qb=qb, g=g, probs=probs, gi=gi, last_group=is_last, bf=bf
                    ):
                        outT_ps, den_ps = get_psums()
                        gtiles, offs = g["tiles"], g["offs"]
                        first = gi == 0
                        n_mm = len(gtiles) - len(g["pairs"])
                        # all PV matmuls, then all den matmuls: fewer psum
                        # accumulation-target switches on the PE
                        for which in ("pv", "den"):
                            emitted = 0
                            for ti, (tl, off) in enumerate(zip(gtiles, offs)):
                                if ti - 1 in g["pairs"]:
                                    continue  # second member of a pair
                                psl = slice(
                                    tl["t0"] * 128, tl["t0"] * 128 + tl["span"]
                                )
                                is_pair = ti in g["pairs"]
                                last = last_group and emitted == n_mm - 1
                                st = first and emitted == 0
                                kj = tl["kj"]
                                tgt = outT_ps if which == "pv" else den_ps
                                if is_pair:
                                    lhs = (
                                        v_sb[:, kj * 128 : (kj + 2) * 128]
                                        if which == "pv"
                                        else ones8_sb[:, 0:256]
                                    ).rearrange("p (two d) -> p two d", two=2)
                                    nc.tensor.matmul(
                                        tgt[:, psl], lhs,
                                        probs[:, off : off + 2 * tl["span"]].rearrange(
                                            "p (two q) -> p two q", two=2
                                        ),
                                        start=st, stop=last,
                                        perf_mode=mybir.MatmulPerfMode.DoubleRow,
                                    )
                                else:
                                    if which == "pv":
                                        lhs = (
                                            t["vb"][:, kj * 128 : (kj + 1) * 128]
                                            if bf
                                            else v_sb[:, kj * 128 : (kj + 1) * 128]
                                        )
                                    else:
                                        lhs = onesb_sb[:] if bf else ones8_sb[:, 0:128]
                                    nc.tensor.matmul(
                                        tgt[:, psl], lhs,
                                        probs[:, off : off + tl["span"]],
                                        start=st, stop=last,
                                    )
                                emitted += 1

                        if last_group:
                            rden = tpool.tile([128, QB], dt.float32, tag="rden")
                            nc.vector.reciprocal_approx_fast(rden[:], den_ps[:])
                            outN = tpool.tile([128, QB], dt.bfloat16, tag="outN")
                            nc.vector.tensor_mul(outN[:], outT_ps[:], rden[:])

                            def tail(u=u, qb=qb, outN=outN):
                                out_sb = tpool.tile([128, QB], dt.bfloat16, tag="out_sb")
                                nc.sync.dma_start_transpose(
                                    out=out_sb[:].rearrange("p (n d) -> p n d", d=128),
                                    in_=outN[:],
                                )
                                nc.sync.dma_start(
                                    out=out[u].rearrange("(n p) d -> p n d", p=128)[
                                        :, qb * NQC : (qb + 1) * NQC, :
                                    ],
                                    in_=out_sb[:].rearrange("p (n d) -> p n d", d=128),
                                )

                            if state["tail"] is not None:
                                state["tail"]()
                            state["tail"] = tail

                    state["pv"].append(pv_emit)
                    if len(state["pv"]) > 2:
                        flush_pv()

            cur = load(0)
            load_consts_rest()
            # next-unit rotary is spread across this unit's query blocks so the
            # PE never idles at unit boundaries
            nxt_rot = [(0, 1024), (1024, 1536), (1536, 2048), (2048, 2560), (2560, 3072)]
            for u in range(per_core):
                nxt = load(u + 1) if u + 1 < per_core else None
                for qb in range(nqb):
                    if u == 0:
                        # rotary one block ahead of attention
                        if qb == 0:
                            rotary(cur, 0, QB)
                        if qb + 1 < nqb:
                            rotary(cur, (qb + 1) * QB, (qb + 2) * QB)
                    if nxt is not None and 1 <= qb <= 5:
                        rotary(nxt, *nxt_rot[qb - 1], pool_add=True)
                    attention_qb(u, cur, qb)
                cur = nxt
            flush_all()
            if state["tail"] is not None:
                state["tail"]()

    nc.compile()
    return nc


def host_prep(q, k, v, cos, sin):
    """Build per-core input maps from full inputs."""
    b, s, h, d = q.shape

    cexp = np.empty((128, s), dtype=np.float32)
    ssig = np.empty((128, s), dtype=np.float32)
    cexp[0::2, :] = cos.T
    cexp[1::2, :] = cos.T
    ssig[0::2, :] = -sin.T
    ssig[1::2, :] = sin.T
    cexp = cexp.astype(BF16)
    ssig = ssig.astype(BF16)

    ones8 = np.ones((128, 256), dtype=FP8)

    # masks in the transposed-score layout: partition p = key offset,
    # column c = query offset
    p = np.arange(128)[:, None]
    c = np.arange(128)[None, :]
    maskD = (c >= p).astype(np.float32)
    w12 = ((c - p) < PARTIAL[12]).astype(np.float32)
    w13 = ((c - p) < PARTIAL[13]).astype(np.float32)

    m0 = np.float32(M0)
    full = np.full((128, 128), m0, dtype=np.float32)
    dpat = np.concatenate([maskD * m0, full, full, full], axis=1).astype(np.float32)
    fwp = np.concatenate([full, full, w12 * m0, w13 * m0], axis=1).astype(np.float32)

    mD8 = maskD.astype(FP8)
    mW12_8 = w12.astype(FP8)
    mW13_8 = w13.astype(FP8)
    mWp8 = np.concatenate([w12, w13], axis=1).astype(FP8)

    units = [(bi, hi) for bi in range(b) for hi in range(h)]
    per = len(units) // N_CORES
    sw = np.arange(128) ^ 1  # row swap: even<->odd
    in_maps = []
    for core in range(N_CORES):
        us = units[core * per : (core + 1) * per]
        qTc = np.stack([q[bi, :, hi, :].T for bi, hi in us]).astype(BF16)
        kTc = np.stack([k[bi, :, hi, :].T for bi, hi in us]).astype(BF16)
        vc = np.stack([v[bi, :, hi, :] for bi, hi in us])
        m = {
            "qT": np.ascontiguousarray(qTc),
            "kT": np.ascontiguousarray(kTc),
            "qTs": np.ascontiguousarray(qTc[:, sw, :]),
            "kTs": np.ascontiguousarray(kTc[:, sw, :]),
            "v8": np.ascontiguousarray(vc.astype(FP8)),
            "vb": np.ascontiguousarray(vc[:, 0:512, :].astype(BF16)),
            "cexpT": cexp,
            "ssigT": ssig,
            "ones8": ones8,
            "onesb": np.ones((128, 128), dtype=BF16),
            "mDb": maskD.astype(BF16),
            "dpat": dpat,
            "fw": fwp,
            "mD": mD8,
            "mW12": mW12_8,
            "mW13": mW13_8,
            "mWp": mWp8,
        }
        in_maps.append(m)
    return in_maps, units


_NC_CACHE = {}


def kernel(q, k, v, cos, sin):
    from concourse.bass_utils import run_bass_kernel_spmd

    q = np.asarray(q, dtype=np.float32)
    k = np.asarray(k, dtype=np.float32)
    v = np.asarray(v, dtype=np.float32)
    cos = np.asarray(cos, dtype=np.float32)
    sin = np.asarray(sin, dtype=np.float32)

    if "nc" not in _NC_CACHE:
        _NC_CACHE["nc"] = build_nc()
    nc = _NC_CACHE["nc"]

    in_maps, units = host_prep(q, k, v, cos, sin)
    res = run_bass_kernel_spmd(nc, in_maps, core_ids=list(range(N_CORES)))

    b, s, h, d = q.shape
    full = np.empty((b, s, h, d), dtype=np.float32)
    per = len(units) // N_CORES
    for core in range(N_CORES):
        o = res.results[core]["out"]  # [per, s, 128] bf16
        for i, (bi, hi) in enumerate(units[core * per : (core + 1) * per]):
            full[bi, :, hi, :] = o[i].astype(np.float32)
    return full


# revision 29
# speedup vs baseline: 1.0359x; 1.0359x over previous
"""Trainium2 Bass kernel for BaseCausalWanSelfAttention (local+sink sparse attention
with interleaved rotary), SPMD across 8 NeuronCores.

Sharding: the 24 (batch, head) pairs are split 3-per-core across 8 cores; each
core runs full local+sink attention for its pairs independently (no collectives).

Design notes (per 128-key x query-span score tile, transposed layout [k, q]):
  - QK matmuls in bf16 (1 cycle/row).
  - probs stored as fp8e4; PV and denominator matmuls run fp8 DoubleRow pairs
    (two adjacent k-tiles contracted per pass) where possible.
  - exp is split: full tiles on the ACT engine (Exp -> fp8 out, batched over
    up to 1536 psum columns); mostly-masked tiles on DVE via a Schraudolph
    bit-trick (one scalar_tensor_tensor: i8 = (score + c0) * pattern, where
    pattern = SCALE*8/ln2 on allowed positions and 0 on masked ones, and the
    int8 bits reinterpret as fp8e4 = approx exp). Masking is free there.
  - rotary uses host-provided row-swapped copies of q/k; the swap-multiply
    runs on GpSimd (Pool), the rest on DVE, all bf16.
  - output transpose via DMA crossbar (dma_start_transpose), not the PE.
"""
import sys

sys.path.insert(0, "/opt/trn_rl_repo")

import numpy as np
import ml_dtypes

import concourse.bacc as bacc
import concourse.tile as tile
import concourse.mybir as mybir

dt = mybir.dt
BF16 = ml_dtypes.bfloat16
FP8 = ml_dtypes.float8_e4m3

# Problem config (hardcoded per contest contract)
B, S, H, D = 2, 3072, 12, 128
LOCAL_WINDOW = 1560
SINK = 128
N_CORES = 8
PER_CORE = (B * H) // N_CORES  # 3
QB = 512
NQC = QB // 128
SCALE = 1.0 / float(np.sqrt(D))

# Schraudolph exp-to-fp8e4 constants: i8 = round((s + C0) * M0) bit-read as
# e4m3 approximates exp(s * SCALE).  M0 = SCALE*8/ln2; C0 = (56 - C)/M0 with
# C=0.45 tuned numerically (rms rel err ~3.1%, bias ~5e-4).
M0 = SCALE * 8.0 / np.log(2.0)
C0 = (56.0 - 0.45) / M0

GROUP_COLS = 1536  # psum columns per score group (3 banks)


def _window_partial_deltas(w):
    out = {}
    for d in range((w - 127 + 127) // 128, (w + 127) // 128 + 1):
        t = w - 128 * d
        if -127 <= t <= 127:
            out[d] = t
    return out


PARTIAL = _window_partial_deltas(LOCAL_WINDOW)  # {12: 24, 13: -104}
W_DELTAS = sorted(PARTIAL)  # [12, 13]
MAX_DELTA = max(PARTIAL)


def chunk_kinds(qb, kj):
    """Valid chunks of k-tile kj for query block qb: list of (t, kind),
    kind in {"full", "diag", ("win", delta)}."""
    kinds = []
    for t in range(NQC):
        qi = NQC * qb + t
        if kj == 0:
            kinds.append((t, "diag" if qi == 0 else "full"))
            continue
        delta = qi - kj
        if delta < 0 or delta > MAX_DELTA:
            continue
        if delta == 0:
            kinds.append((t, "diag"))
        elif delta in PARTIAL:
            kinds.append((t, ("win", delta)))
        else:
            kinds.append((t, "full"))
    return kinds


def kj_list(qb):
    n_ktiles = S // 128
    hi = min(NQC * qb + NQC - 1, n_ktiles - 1)
    lo = max(1, NQC * qb - MAX_DELTA)
    return [0] + list(range(lo, hi + 1))


def plan_qb(qb):
    """Plan tiles + groups for a query block.

    Tile: dict(kj, t0, span, exp in {"act","stt"}, pat, post)
      pat  (stt): ("dpat"|"fw", lo, hi) slice of the pattern const
      post (act): (c_lo, c_hi, maskname) columns within the tile to zero after
    Group: dict(tiles=[...], offs=[...], act_cols=n, pairs=[i...])
      pair indices are positions of DoubleRow pair starts in tiles[].

    qb 0 runs fully in bf16 (ACT exp + post masks): its softmax sees few keys,
    so fp8 quantization error would pass straight into the output there.
    """
    tiles = []
    for kj in kj_list(qb):
        kinds = chunk_kinds(qb, kj)
        assert kinds
        t0, t1 = kinds[0][0], kinds[-1][0] + 1
        span = (t1 - t0) * 128
        masked = [(t, k) for t, k in kinds if k != "full"]
        tl = dict(kj=kj, t0=t0, span=span, post=None, pat=None)
        if not masked:
            tl["exp"] = "act"
        elif qb == 0:
            tl["exp"] = "act"
            assert masked[0][1] == "diag" and masked[0][0] == t0, (qb, kj)
            tl["post"] = (0, 128, "mD")
        elif span <= 384:
            tl["exp"] = "stt"
            if kinds[0][1] == "diag":
                # diag-first pattern [D, F, F, ...]
                assert all(k == "full" for _, k in kinds[1:])
                tl["pat"] = ("dpat", 0, span)
            else:
                # window-last pattern: suffix of [F, F, W12, W13]
                tl["pat"] = ("fw", 512 - span, 512)
        else:
            tl["exp"] = "act"
            # masked chunks are a contiguous prefix (diag) or suffix (win)
            mts = [t - t0 for t, _ in masked]
            kindnames = [k if isinstance(k, str) else f"w{k[1]}" for _, k in masked]
            if masked[0][1] == "diag":
                assert mts == [0]
                tl["post"] = (0, 128, "mD")
            else:
                c_lo = mts[0] * 128
                assert mts == list(range(mts[0], mts[0] + len(mts)))
                if kindnames == ["w12", "w13"]:
                    tl["post"] = (c_lo, c_lo + 256, "mWp")
                elif kindnames == ["w12"]:
                    tl["post"] = (c_lo, c_lo + 128, "mW12")
                elif kindnames == ["w13"]:
                    tl["post"] = (c_lo, c_lo + 128, "mW13")
                else:
                    raise AssertionError((qb, kj, kindnames))
        tiles.append(tl)

    # Pair adjacent-kj ACT tiles with identical (t0, span) for DoubleRow.
    act_idx = [i for i, t in enumerate(tiles) if t["exp"] == "act"]
    paired = {}
    i = 0
    while i + 1 < len(act_idx):
        a, b = act_idx[i], act_idx[i + 1]
        ta, tb = tiles[a], tiles[b]
        if (
            tb["kj"] == ta["kj"] + 1
            and ta["t0"] == tb["t0"]
            and ta["span"] == tb["span"]
        ):
            paired[a] = b
            i += 2
        else:
            i += 1

    # Pack into groups of <= GROUP_COLS/512 psum bank slots.  Every tile gets
    # a 512-aligned psum slot (a matmul output must not cross a 2KB psum bank
    # boundary); a pair takes two adjacent slots.
    units = []
    used = set()
    for a, b in paired.items():
        units.append([a, b])
        used.update((a, b))
    for i, t in enumerate(tiles):
        if i not in used:
            units.append([i])
    n_slots = GROUP_COLS // 512
    groups = []
    cur, slots = [], 0
    for u in units:
        if cur and slots + len(u) > n_slots:
            groups.append(cur)
            cur, slots = [], 0
        cur.append(u)
        slots += len(u)
    if cur:
        groups.append(cur)

    out = []
    for g in groups:
        gtiles, offs, pairs = [], [], []
        slot = 0
        # act tiles first within the group so the exp ranges merge
        for u in sorted(g, key=lambda u: 0 if tiles[u[0]]["exp"] == "act" else 1):
            if len(u) == 2:
                pairs.append(len(gtiles))
            for i in u:
                gtiles.append(tiles[i])
                offs.append(slot * 512)
                slot += 1
        # maximal contiguous psum ranges covering the act tiles (for exp)
        act_segs = []
        for tl, off in zip(gtiles, offs):
            if tl["exp"] != "act":
                continue
            if act_segs and act_segs[-1][1] == off:
                act_segs[-1][1] = off + tl["span"]
            else:
                act_segs.append([off, off + tl["span"]])
        out.append(dict(tiles=gtiles, offs=offs, act_segs=act_segs, pairs=pairs))
    return out


def build_nc(per_core=PER_CORE):
    nqb = S // QB
    nc = bacc.Bacc("TRN2", target_bir_lowering=False, debug=False)

    qT = nc.declare_dram_parameter("qT", [per_core, 128, S], dt.bfloat16, isOutput=False)
    kT = nc.declare_dram_parameter("kT", [per_core, 128, S], dt.bfloat16, isOutput=False)
    qTs = nc.declare_dram_parameter("qTs", [per_core, 128, S], dt.bfloat16, isOutput=False)
    kTs = nc.declare_dram_parameter("kTs", [per_core, 128, S], dt.bfloat16, isOutput=False)
    v8 = nc.declare_dram_parameter("v8", [per_core, S, 128], dt.float8e4, isOutput=False)
    vb = nc.declare_dram_parameter("vb", [per_core, 512, 128], dt.bfloat16, isOutput=False)
    cexpT = nc.declare_dram_parameter("cexpT", [128, S], dt.bfloat16, isOutput=False)
    ssigT = nc.declare_dram_parameter("ssigT", [128, S], dt.bfloat16, isOutput=False)
    ones8 = nc.declare_dram_parameter("ones8", [128, 256], dt.float8e4, isOutput=False)
    onesb = nc.declare_dram_parameter("onesb", [128, 128], dt.bfloat16, isOutput=False)
    mDb = nc.declare_dram_parameter("mDb", [128, 128], dt.bfloat16, isOutput=False)
    dpat = nc.declare_dram_parameter("dpat", [128, 512], dt.float32, isOutput=False)
    fw = nc.declare_dram_parameter("fw", [128, 512], dt.float32, isOutput=False)
    mD = nc.declare_dram_parameter("mD", [128, 128], dt.float8e4, isOutput=False)
    mW12 = nc.declare_dram_parameter("mW12", [128, 128], dt.float8e4, isOutput=False)
    mW13 = nc.declare_dram_parameter("mW13", [128, 128], dt.float8e4, isOutput=False)
    mWp = nc.declare_dram_parameter("mWp", [128, 256], dt.float8e4, isOutput=False)
    out = nc.declare_dram_parameter("out", [per_core, S, 128], dt.bfloat16, isOutput=True)

    with tile.TileContext(nc) as tc:
        with (
            tc.tile_pool(name="const", bufs=1) as cpool,
            tc.tile_pool(name="big", bufs=2) as bigpool,
            tc.tile_pool(name="probs", bufs=4) as ppool,
            tc.tile_pool(name="tail", bufs=2) as tpool,
            tc.tile_pool(name="ps_sc", bufs=2, space="PSUM") as ps_sc,
            tc.tile_pool(name="ps_out", bufs=1, space="PSUM") as ps_out,
            tc.tile_pool(name="ps_den", bufs=1, space="PSUM") as ps_den,
        ):
            # constants
            cexp_sb = cpool.tile([128, S], dt.bfloat16, tag="cexp")
            ssig_sb = cpool.tile([128, S], dt.bfloat16, tag="ssig")
            nc.sync.dma_start(out=cexp_sb[:, 0:1024], in_=cexpT[:, 0:1024])
            nc.sync.dma_start(out=ssig_sb[:, 0:1024], in_=ssigT[:, 0:1024])
            ones8_sb = cpool.tile([128, 256], dt.float8e4, tag="ones8")
            onesb_sb = cpool.tile([128, 128], dt.bfloat16, tag="onesb")
            mDb_sb = cpool.tile([128, 128], dt.bfloat16, tag="mDb")
            dpat_sb = cpool.tile([128, 512], dt.float32, tag="dpat")
            fw_sb = cpool.tile([128, 512], dt.float32, tag="fw")
            mask_sb = {
                "mD": cpool.tile([128, 128], dt.float8e4, tag="mD", name="mD"),
                "mW12": cpool.tile([128, 128], dt.float8e4, tag="mW12", name="mW12"),
                "mW13": cpool.tile([128, 128], dt.float8e4, tag="mW13", name="mW13"),
                "mWp": cpool.tile([128, 256], dt.float8e4, tag="mWp", name="mWp"),
            }
            pat_sb = {"dpat": dpat_sb, "fw": fw_sb}

            def load_consts_rest():
                nc.sync.dma_start(out=ones8_sb[:], in_=ones8[:])
                nc.sync.dma_start(out=onesb_sb[:], in_=onesb[:])
                nc.sync.dma_start(out=mDb_sb[:], in_=mDb[:])
                nc.sync.dma_start(out=dpat_sb[:], in_=dpat[:])
                nc.sync.dma_start(out=fw_sb[:], in_=fw[:])
                nc.sync.dma_start(out=mask_sb["mD"][:], in_=mD[:])
                nc.sync.dma_start(out=mask_sb["mW12"][:], in_=mW12[:])
                nc.sync.dma_start(out=mask_sb["mW13"][:], in_=mW13[:])
                nc.sync.dma_start(out=mask_sb["mWp"][:], in_=mWp[:])
                for c2 in range(1, S // 1024):
                    sl2 = slice(c2 * 1024, (c2 + 1) * 1024)
                    nc.sync.dma_start(out=cexp_sb[:, sl2], in_=cexpT[:, sl2])
                    nc.sync.dma_start(out=ssig_sb[:, sl2], in_=ssigT[:, sl2])

            def load(u):
                qraw = bigpool.tile([128, S], dt.bfloat16, tag="qraw")
                kraw = bigpool.tile([128, S], dt.bfloat16, tag="kraw")
                qsw = bigpool.tile([128, S], dt.bfloat16, tag="qsw")
                ksw = bigpool.tile([128, S], dt.bfloat16, tag="ksw")
                v_sb = bigpool.tile([128, S], dt.float8e4, tag="v8")
                vb_sb = bigpool.tile([128, 512], dt.bfloat16, tag="vb")
                # head chunks first so rotary/QK of the first block can start
                # while the rest streams in
                H0 = 1024
                for tsb, tdr in ((qraw, qT), (kraw, kT), (qsw, qTs), (ksw, kTs)):
                    nc.sync.dma_start(out=tsb[:, 0:H0], in_=tdr[u][:, 0:H0])
                nc.sync.dma_start(
                    out=vb_sb[:].rearrange("p (n d) -> p n d", d=128),
                    in_=vb[u].rearrange("(n p) d -> p n d", p=128),
                )
                nc.sync.dma_start(
                    out=v_sb[:].rearrange("p (n d) -> p n d", d=128),
                    in_=v8[u].rearrange("(n p) d -> p n d", p=128),
                )
                for tsb, tdr in ((qraw, qT), (kraw, kT), (qsw, qTs), (ksw, kTs)):
                    nc.sync.dma_start(out=tsb[:, H0:S], in_=tdr[u][:, H0:S])
                rq = bigpool.tile([128, S], dt.bfloat16, tag="rq")
                rk = bigpool.tile([128, S], dt.bfloat16, tag="rk")
                swm = bigpool.tile([128, S], dt.bfloat16, tag="swm")
                return dict(
                    q=qraw, k=kraw, qs=qsw, ks=ksw, v=v_sb, vb=vb_sb,
                    rq=rq, rk=rk, sw=swm,
                )

            def rotary(t, lo, hi, pool_add=False):
                """rq = q*cexp + qsw*ssig on [lo,hi); swap-mult on Pool.
                pool_add also moves the final add to Pool (for the next-unit
                rotary that runs while this unit's attention keeps DVE busy)."""
                step = 1024 if (hi - lo) % 1024 == 0 else 512
                for raw, swr, r in ((t["q"], t["qs"], t["rq"]), (t["k"], t["ks"], t["rk"])):
                    for c in range(lo // step, hi // step):
                        sl = slice(c * step, (c + 1) * step)
                        nc.gpsimd.tensor_mul(t["sw"][:, sl], swr[:, sl], ssig_sb[:, sl])
                        nc.vector.tensor_mul(r[:, sl], raw[:, sl], cexp_sb[:, sl])
                        if pool_add:
                            nc.gpsimd.tensor_add(r[:, sl], r[:, sl], t["sw"][:, sl])
                        else:
                            nc.vector.tensor_add(r[:, sl], r[:, sl], t["sw"][:, sl])

            state = {"pv": [], "tail": None}

            def flush_pv():
                if state["pv"]:
                    state["pv"].pop(0)()

            def flush_all():
                while state["pv"]:
                    flush_pv()

            def attention_qb(u, t, qb):
                groups = plan_qb(qb)
                n_groups = len(groups)
                rq, rk, v_sb = t["rq"], t["rk"], t["v"]
                qbctx = {}

                def get_psums():
                    if "outT" not in qbctx:
                        qbctx["outT"] = ps_out.tile(
                            [128, QB], dt.float32, tag="outT", name="outT"
                        )
                        qbctx["den"] = ps_den.tile(
                            [128, QB], dt.float32, tag="den", name="den"
                        )
                    return qbctx["outT"], qbctx["den"]

                bf = qb == 0  # query-block 0 runs in bf16 (see plan_qb)
                for gi, g in enumerate(groups):
                    gtiles, offs = g["tiles"], g["offs"]
                    if bf:
                        probs = ppool.tile([128, GROUP_COLS], dt.bfloat16, tag="probsb")
                    else:
                        probs = ppool.tile([128, GROUP_COLS], dt.float8e4, tag="probs")
                    sc = ps_sc.tile([128, GROUP_COLS], dt.float32, tag="sc")
                    # QK for every tile in the group
                    for tl, off in zip(gtiles, offs):
                        csl = slice(qb * QB + tl["t0"] * 128, qb * QB + tl["t0"] * 128 + tl["span"])
                        ksl = slice(tl["kj"] * 128, (tl["kj"] + 1) * 128)
                        nc.tensor.matmul(
                            sc[:, off : off + tl["span"]],
                            rk[:, ksl], rq[:, csl], start=True, stop=True,
                        )
                    # exp: one ACT instruction per contiguous act psum range
                    for lo, hi in g["act_segs"]:
                        nc.scalar.activation(
                            probs[:, lo:hi],
                            sc[:, lo:hi],
                            mybir.ActivationFunctionType.Exp,
                            scale=SCALE,
                        )
                    # stt tiles: Schraudolph exp+mask in one DVE op each
                    for tl, off in zip(gtiles, offs):
                        if tl["exp"] == "stt":
                            pname, plo, phi = tl["pat"]
                            nc.vector.scalar_tensor_tensor(
                                out=probs[:, off : off + tl["span"]].bitcast(dt.int8),
                                in0=sc[:, off : off + tl["span"]],
                                scalar=C0,
                                in1=pat_sb[pname][:, plo:phi],
                                op0=mybir.AluOpType.add,
                                op1=mybir.AluOpType.mult,
                            )
                    # post-masks for act tiles with masked chunks
                    for tl, off in zip(gtiles, offs):
                        if tl["post"] is not None:
                            c_lo, c_hi, mn = tl["post"]
                            m = mDb_sb if bf else mask_sb[mn]
                            assert not bf or mn == "mD"
                            nc.vector.tensor_mul(
                                probs[:, off + c_lo : off + c_hi],
                                probs[:, off + c_lo : off + c_hi],
                                m[:],
                            )

                    is_last = gi == n_groups - 1

                    def pv_emit(
                        u=u, qb=qb, g=g, probs=probs, gi=gi, last_group=is_last, bf=bf
                    ):
                        outT_ps, den_ps = get_psums()
                        gtiles, offs = g["tiles"], g["offs"]
                        first = gi == 0
                        n_mm = len(gtiles) - len(g["pairs"])
                        # all PV matmuls, then all den matmuls: fewer psum
                        # accumulation-target switches on the PE
                        for which in ("pv", "den"):
                            emitted = 0
                            for ti, (tl, off) in enumerate(zip(gtiles, offs)):
                                if ti - 1 in g["pairs"]:
                                    continue  # second member of a pair
                                psl = slice(
                                    tl["t0"] * 128, tl["t0"] * 128 + tl["span"]
                                )
                                is_pair = ti in g["pairs"]
                                last = last_group and emitted == n_mm - 1
                                st = first and emitted == 0
                                kj = tl["kj"]
                                tgt = outT_ps if which == "pv" else den_ps
                                if is_pair:
                                    lhs = (
                                        v_sb[:, kj * 128 : (kj + 2) * 128]
                                        if which == "pv"
                                        else ones8_sb[:, 0:256]
                                    ).rearrange("p (two d) -> p two d", two=2)
                                    nc.tensor.matmul(
                                        tgt[:, psl], lhs,
                                        probs[:, off : off + 2 * tl["span"]].rearrange(
                                            "p (two q) -> p two q", two=2
                                        ),
                                        start=st, stop=last,
                                        perf_mode=mybir.MatmulPerfMode.DoubleRow,
                                    )
                                else:
                                    if which == "pv":
                                        lhs = (
                                            t["vb"][:, kj * 128 : (kj + 1) * 128]
                                            if bf
                                            else v_sb[:, kj * 128 : (kj + 1) * 128]
                                        )
                                    else:
                                        lhs = onesb_sb[:] if bf else ones8_sb[:, 0:128]
                                    nc.tensor.matmul(
                                        tgt[:, psl], lhs,
                                        probs[:, off : off + tl["span"]],
                                        start=st, stop=last,
                                    )
                                emitted += 1

                        if last_group:
                            rden = tpool.tile([128, QB], dt.float32, tag="rden")
                            nc.vector.reciprocal_approx_fast(rden[:], den_ps[:])
                            outN = tpool.tile([128, QB], dt.bfloat16, tag="outN")
                            nc.vector.tensor_mul(outN[:], outT_ps[:], rden[:])

                            def tail(u=u, qb=qb, outN=outN):
                                out_sb = tpool.tile([128, QB], dt.bfloat16, tag="out_sb")
                                nc.sync.dma_start_transpose(
                                    out=out_sb[:].rearrange("p (n d) -> p n d", d=128),
                                    in_=outN[:],
                                )
                                nc.sync.dma_start(
                                    out=out[u].rearrange("(n p) d -> p n d", p=128)[
                                        :, qb * NQC : (qb + 1) * NQC, :
                                    ],
                                    in_=out_sb[:].rearrange("p (n d) -> p n d", d=128),
                                )

                            if state["tail"] is not None:
                                state["tail"]()
                            state["tail"] = tail

                    state["pv"].append(pv_emit)
                    if len(state["pv"]) > 2:
                        flush_pv()

            cur = load(0)
            load_consts_rest()
            # rotary runs lazily one query-block ahead of attention in every
            # unit, spreading the DVE load evenly
            for u in range(per_core):
                nxt = load(u + 1) if u + 1 < per_core else None
                for qb in range(nqb):
                    if qb == 0:
                        rotary(cur, 0, QB)
                    if qb + 1 < nqb:
                        rotary(cur, (qb + 1) * QB, (qb + 2) * QB)
                    attention_qb(u, cur, qb)
                cur = nxt
            flush_all()
            if state["tail"] is not None:
                state["tail"]()

    nc.compile()
    return nc


def host_prep(q, k, v, cos, sin):
    """Build per-core input maps from full inputs."""
    b, s, h, d = q.shape

    cexp = np.empty((128, s), dtype=np.float32)
    ssig = np.empty((128, s), dtype=np.float32)
    cexp[0::2, :] = cos.T
    cexp[1::2, :] = cos.T
    ssig[0::2, :] = -sin.T
    ssig[1::2, :] = sin.T
    cexp = cexp.astype(BF16)
    ssig = ssig.astype(BF16)

    ones8 = np.ones((128, 256), dtype=FP8)

    # masks in the transposed-score layout: partition p = key offset,
    # column c = query offset
    p = np.arange(128)[:, None]
    c = np.arange(128)[None, :]
    maskD = (c >= p).astype(np.float32)
    w12 = ((c - p) < PARTIAL[12]).astype(np.float32)
    w13 = ((c - p) < PARTIAL[13]).astype(np.float32)

    m0 = np.float32(M0)
    full = np.full((128, 128), m0, dtype=np.float32)
    dpat = np.concatenate([maskD * m0, full, full, full], axis=1).astype(np.float32)
    fwp = np.concatenate([full, full, w12 * m0, w13 * m0], axis=1).astype(np.float32)

    mD8 = maskD.astype(FP8)
    mW12_8 = w12.astype(FP8)
    mW13_8 = w13.astype(FP8)
    mWp8 = np.concatenate([w12, w13], axis=1).astype(FP8)

    units = [(bi, hi) for bi in range(b) for hi in range(h)]
    per = len(units) // N_CORES
    sw = np.arange(128) ^ 1  # row swap: even<->odd
    in_maps = []
    for core in range(N_CORES):
        us = units[core * per : (core + 1) * per]
        qTc = np.stack([q[bi, :, hi, :].T for bi, hi in us]).astype(BF16)
        kTc = np.stack([k[bi, :, hi, :].T for bi, hi in us]).astype(BF16)
        vc = np.stack([v[bi, :, hi, :] for bi, hi in us])
        m = {
            "qT": np.ascontiguousarray(qTc),
            "kT": np.ascontiguousarray(kTc),
            "qTs": np.ascontiguousarray(qTc[:, sw, :]),
            "kTs": np.ascontiguousarray(kTc[:, sw, :]),
            "v8": np.ascontiguousarray(vc.astype(FP8)),
            "vb": np.ascontiguousarray(vc[:, 0:512, :].astype(BF16)),
            "cexpT": cexp,
            "ssigT": ssig,
            "ones8": ones8,
            "onesb": np.ones((128, 128), dtype=BF16),
            "mDb": maskD.astype(BF16),
            "dpat": dpat,
            "fw": fwp,
            "mD": mD8,
            "mW12": mW12_8,
            "mW13": mW13_8,
            "mWp": mWp8,
        }
        in_maps.append(m)
    return in_maps, units


_NC_CACHE = {}


def kernel(q, k, v, cos, sin):
    from concourse.bass_utils import run_bass_kernel_spmd

    q = np.asarray(q, dtype=np.float32)
    k = np.asarray(k, dtype=np.float32)
    v = np.asarray(v, dtype=np.float32)
    cos = np.asarray(cos, dtype=np.float32)
    sin = np.asarray(sin, dtype=np.float32)

    if "nc" not in _NC_CACHE:
        _NC_CACHE["nc"] = build_nc()
    nc = _NC_CACHE["nc"]

    in_maps, units = host_prep(q, k, v, cos, sin)
    res = run_bass_kernel_spmd(nc, in_maps, core_ids=list(range(N_CORES)))

    b, s, h, d = q.shape
    full = np.empty((b, s, h, d), dtype=np.float32)
    per = len(units) // N_CORES
    for core in range(N_CORES):
        o = res.results[core]["out"]  # [per, s, 128] bf16
        for i, (bi, hi) in enumerate(units[core * per : (core + 1) * per]):
            full[bi, :, hi, :] = o[i].astype(np.float32)
    return full


# revision 31
# speedup vs baseline: 1.0566x; 1.0200x over previous
"""Trainium2 Bass kernel for BaseCausalWanSelfAttention (local+sink sparse attention
with interleaved rotary), SPMD across 8 NeuronCores.

Sharding: the 24 (batch, head) pairs are split 3-per-core across 8 cores; each
core runs full local+sink attention for its pairs independently (no collectives).

Design notes (per 128-key x query-span score tile, transposed layout [k, q]):
  - QK matmuls in bf16 (1 cycle/row).
  - probs stored as fp8e4; PV and denominator matmuls run fp8 DoubleRow pairs
    (two adjacent k-tiles contracted per pass) where possible.
  - exp is split: full tiles on the ACT engine (Exp -> fp8 out, batched over
    up to 1536 psum columns); mostly-masked tiles on DVE via a Schraudolph
    bit-trick (one scalar_tensor_tensor: i8 = (score + c0) * pattern, where
    pattern = SCALE*8/ln2 on allowed positions and 0 on masked ones, and the
    int8 bits reinterpret as fp8e4 = approx exp). Masking is free there.
  - rotary uses host-provided row-swapped copies of q/k; the swap-multiply
    runs on GpSimd (Pool), the rest on DVE, all bf16.
  - output transpose via DMA crossbar (dma_start_transpose), not the PE.
"""
import sys

sys.path.insert(0, "/opt/trn_rl_repo")

import numpy as np
import ml_dtypes

import concourse.bacc as bacc
import concourse.tile as tile
import concourse.mybir as mybir

dt = mybir.dt
BF16 = ml_dtypes.bfloat16
FP8 = ml_dtypes.float8_e4m3

# Problem config (hardcoded per contest contract)
B, S, H, D = 2, 3072, 12, 128
LOCAL_WINDOW = 1560
SINK = 128
N_CORES = 8
PER_CORE = (B * H) // N_CORES  # 3
QB = 512
NQC = QB // 128
SCALE = 1.0 / float(np.sqrt(D))

# Schraudolph exp-to-fp8e4 constants: i8 = round((s + C0) * M0) bit-read as
# e4m3 approximates exp(s * SCALE).  M0 = SCALE*8/ln2; C0 = (56 - C)/M0 with
# C=0.45 tuned numerically (rms rel err ~3.1%, bias ~5e-4).
M0 = SCALE * 8.0 / np.log(2.0)
C0 = (56.0 - 0.45) / M0

GROUP_COLS = 1536  # psum columns per score group (3 banks)


def _window_partial_deltas(w):
    out = {}
    for d in range((w - 127 + 127) // 128, (w + 127) // 128 + 1):
        t = w - 128 * d
        if -127 <= t <= 127:
            out[d] = t
    return out


PARTIAL = _window_partial_deltas(LOCAL_WINDOW)  # {12: 24, 13: -104}
W_DELTAS = sorted(PARTIAL)  # [12, 13]
MAX_DELTA = max(PARTIAL)


def chunk_kinds(qb, kj):
    """Valid chunks of k-tile kj for query block qb: list of (t, kind),
    kind in {"full", "diag", ("win", delta)}."""
    kinds = []
    for t in range(NQC):
        qi = NQC * qb + t
        if kj == 0:
            kinds.append((t, "diag" if qi == 0 else "full"))
            continue
        delta = qi - kj
        if delta < 0 or delta > MAX_DELTA:
            continue
        if delta == 0:
            kinds.append((t, "diag"))
        elif delta in PARTIAL:
            kinds.append((t, ("win", delta)))
        else:
            kinds.append((t, "full"))
    return kinds


def kj_list(qb):
    n_ktiles = S // 128
    hi = min(NQC * qb + NQC - 1, n_ktiles - 1)
    lo = max(1, NQC * qb - MAX_DELTA)
    return [0] + list(range(lo, hi + 1))


def plan_qb(qb):
    """Plan tiles + groups for a query block.

    Tile: dict(kj, t0, span, exp in {"act","stt"}, pat, post)
      pat  (stt): ("dpat"|"fw", lo, hi) slice of the pattern const
      post (act): (c_lo, c_hi, maskname) columns within the tile to zero after
    Group: dict(tiles=[...], offs=[...], act_cols=n, pairs=[i...])
      pair indices are positions of DoubleRow pair starts in tiles[].

    qb 0 runs fully in bf16 (ACT exp + post masks): its softmax sees few keys,
    so fp8 quantization error would pass straight into the output there.
    """
    tiles = []
    for kj in kj_list(qb):
        kinds = chunk_kinds(qb, kj)
        assert kinds
        t0, t1 = kinds[0][0], kinds[-1][0] + 1
        span = (t1 - t0) * 128
        masked = [(t, k) for t, k in kinds if k != "full"]
        tl = dict(kj=kj, t0=t0, span=span, post=None, pat=None)
        if not masked:
            tl["exp"] = "act"
        elif qb == 0:
            tl["exp"] = "act"
            assert masked[0][1] == "diag" and masked[0][0] == t0, (qb, kj)
            tl["post"] = (0, 128, "mD")
        elif span <= 384:
            tl["exp"] = "stt"
            if kinds[0][1] == "diag":
                # diag-first pattern [D, F, F, ...]
                assert all(k == "full" for _, k in kinds[1:])
                tl["pat"] = ("dpat", 0, span)
            else:
                # window-last pattern: suffix of [F, F, W12, W13]
                tl["pat"] = ("fw", 512 - span, 512)
        else:
            tl["exp"] = "act"
            # masked chunks are a contiguous prefix (diag) or suffix (win)
            mts = [t - t0 for t, _ in masked]
            kindnames = [k if isinstance(k, str) else f"w{k[1]}" for _, k in masked]
            if masked[0][1] == "diag":
                assert mts == [0]
                tl["post"] = (0, 128, "mD")
            else:
                c_lo = mts[0] * 128
                assert mts == list(range(mts[0], mts[0] + len(mts)))
                if kindnames == ["w12", "w13"]:
                    tl["post"] = (c_lo, c_lo + 256, "mWp")
                elif kindnames == ["w12"]:
                    tl["post"] = (c_lo, c_lo + 128, "mW12")
                elif kindnames == ["w13"]:
                    tl["post"] = (c_lo, c_lo + 128, "mW13")
                else:
                    raise AssertionError((qb, kj, kindnames))
        tiles.append(tl)

    # Pair adjacent-kj ACT tiles with identical (t0, span) for DoubleRow.
    act_idx = [i for i, t in enumerate(tiles) if t["exp"] == "act"]
    paired = {}
    i = 0
    while i + 1 < len(act_idx):
        a, b = act_idx[i], act_idx[i + 1]
        ta, tb = tiles[a], tiles[b]
        if (
            tb["kj"] == ta["kj"] + 1
            and ta["t0"] == tb["t0"]
            and ta["span"] == tb["span"]
        ):
            paired[a] = b
            i += 2
        else:
            i += 1

    # Pack into groups of <= GROUP_COLS/512 psum bank slots.  Every tile gets
    # a 512-aligned psum slot (a matmul output must not cross a 2KB psum bank
    # boundary); a pair takes two adjacent slots.
    units = []
    used = set()
    for a, b in paired.items():
        units.append([a, b])
        used.update((a, b))
    for i, t in enumerate(tiles):
        if i not in used:
            units.append([i])
    n_slots = GROUP_COLS // 512
    groups = []
    cur, slots = [], 0
    for u in units:
        if cur and slots + len(u) > n_slots:
            groups.append(cur)
            cur, slots = [], 0
        cur.append(u)
        slots += len(u)
    if cur:
        groups.append(cur)

    out = []
    for g in groups:
        gtiles, offs, pairs = [], [], []
        slot = 0
        # act tiles first within the group so the exp ranges merge
        for u in sorted(g, key=lambda u: 0 if tiles[u[0]]["exp"] == "act" else 1):
            if len(u) == 2:
                pairs.append(len(gtiles))
            for i in u:
                gtiles.append(tiles[i])
                offs.append(slot * 512)
                slot += 1
        # maximal contiguous psum ranges covering the act tiles (for exp)
        act_segs = []
        for tl, off in zip(gtiles, offs):
            if tl["exp"] != "act":
                continue
            if act_segs and act_segs[-1][1] == off:
                act_segs[-1][1] = off + tl["span"]
            else:
                act_segs.append([off, off + tl["span"]])
        out.append(dict(tiles=gtiles, offs=offs, act_segs=act_segs, pairs=pairs))
    return out


def build_nc(per_core=PER_CORE):
    nqb = S // QB
    nc = bacc.Bacc("TRN2", target_bir_lowering=False, debug=False)

    qT = nc.declare_dram_parameter("qT", [per_core, 128, S], dt.bfloat16, isOutput=False)
    kT = nc.declare_dram_parameter("kT", [per_core, 128, S], dt.bfloat16, isOutput=False)
    qTs = nc.declare_dram_parameter("qTs", [per_core, 128, S], dt.bfloat16, isOutput=False)
    kTs = nc.declare_dram_parameter("kTs", [per_core, 128, S], dt.bfloat16, isOutput=False)
    v8 = nc.declare_dram_parameter("v8", [per_core, S, 128], dt.float8e4, isOutput=False)
    vb = nc.declare_dram_parameter("vb", [per_core, 512, 128], dt.bfloat16, isOutput=False)
    cexpT = nc.declare_dram_parameter("cexpT", [128, S], dt.bfloat16, isOutput=False)
    ssigT = nc.declare_dram_parameter("ssigT", [128, S], dt.bfloat16, isOutput=False)
    ones8 = nc.declare_dram_parameter("ones8", [128, 256], dt.float8e4, isOutput=False)
    onesb = nc.declare_dram_parameter("onesb", [128, 128], dt.bfloat16, isOutput=False)
    mDb = nc.declare_dram_parameter("mDb", [128, 128], dt.bfloat16, isOutput=False)
    dpat = nc.declare_dram_parameter("dpat", [128, 512], dt.float32, isOutput=False)
    fw = nc.declare_dram_parameter("fw", [128, 512], dt.float32, isOutput=False)
    mD = nc.declare_dram_parameter("mD", [128, 128], dt.float8e4, isOutput=False)
    mW12 = nc.declare_dram_parameter("mW12", [128, 128], dt.float8e4, isOutput=False)
    mW13 = nc.declare_dram_parameter("mW13", [128, 128], dt.float8e4, isOutput=False)
    mWp = nc.declare_dram_parameter("mWp", [128, 256], dt.float8e4, isOutput=False)
    out = nc.declare_dram_parameter("out", [per_core, S, 128], dt.bfloat16, isOutput=True)

    with tile.TileContext(nc) as tc:
        with (
            tc.tile_pool(name="const", bufs=1) as cpool,
            tc.tile_pool(name="big", bufs=2) as bigpool,
            tc.tile_pool(name="probs", bufs=4) as ppool,
            tc.tile_pool(name="tail", bufs=2) as tpool,
            tc.tile_pool(name="ps_sc", bufs=2, space="PSUM") as ps_sc,
            tc.tile_pool(name="ps_out", bufs=1, space="PSUM") as ps_out,
            tc.tile_pool(name="ps_den", bufs=1, space="PSUM") as ps_den,
        ):
            # constants
            cexp_sb = cpool.tile([128, S], dt.bfloat16, tag="cexp")
            ssig_sb = cpool.tile([128, S], dt.bfloat16, tag="ssig")
            nc.sync.dma_start(out=cexp_sb[:, 0:1024], in_=cexpT[:, 0:1024])
            nc.sync.dma_start(out=ssig_sb[:, 0:1024], in_=ssigT[:, 0:1024])
            ones8_sb = cpool.tile([128, 256], dt.float8e4, tag="ones8")
            onesb_sb = cpool.tile([128, 128], dt.bfloat16, tag="onesb")
            mDb_sb = cpool.tile([128, 128], dt.bfloat16, tag="mDb")
            dpat_sb = cpool.tile([128, 512], dt.float32, tag="dpat")
            fw_sb = cpool.tile([128, 512], dt.float32, tag="fw")
            mask_sb = {
                "mD": cpool.tile([128, 128], dt.float8e4, tag="mD", name="mD"),
                "mW12": cpool.tile([128, 128], dt.float8e4, tag="mW12", name="mW12"),
                "mW13": cpool.tile([128, 128], dt.float8e4, tag="mW13", name="mW13"),
                "mWp": cpool.tile([128, 256], dt.float8e4, tag="mWp", name="mWp"),
            }
            pat_sb = {"dpat": dpat_sb, "fw": fw_sb}

            def load_consts_rest():
                nc.sync.dma_start(out=ones8_sb[:], in_=ones8[:])
                nc.sync.dma_start(out=onesb_sb[:], in_=onesb[:])
                nc.sync.dma_start(out=mDb_sb[:], in_=mDb[:])
                nc.sync.dma_start(out=dpat_sb[:], in_=dpat[:])
                nc.sync.dma_start(out=fw_sb[:], in_=fw[:])
                nc.sync.dma_start(out=mask_sb["mD"][:], in_=mD[:])
                nc.sync.dma_start(out=mask_sb["mW12"][:], in_=mW12[:])
                nc.sync.dma_start(out=mask_sb["mW13"][:], in_=mW13[:])
                nc.sync.dma_start(out=mask_sb["mWp"][:], in_=mWp[:])
                for c2 in range(1, S // 1024):
                    sl2 = slice(c2 * 1024, (c2 + 1) * 1024)
                    nc.sync.dma_start(out=cexp_sb[:, sl2], in_=cexpT[:, sl2])
                    nc.sync.dma_start(out=ssig_sb[:, sl2], in_=ssigT[:, sl2])

            def load(u):
                qraw = bigpool.tile([128, S], dt.bfloat16, tag="qraw")
                kraw = bigpool.tile([128, S], dt.bfloat16, tag="kraw")
                qsw = bigpool.tile([128, S], dt.bfloat16, tag="qsw")
                ksw = bigpool.tile([128, S], dt.bfloat16, tag="ksw")
                v_sb = bigpool.tile([128, S], dt.float8e4, tag="v8")
                vb_sb = bigpool.tile([128, 512], dt.bfloat16, tag="vb")
                # head chunks first so rotary/QK of the first block can start
                # while the rest streams in
                H0 = 512
                for tsb, tdr in ((qraw, qT), (kraw, kT), (qsw, qTs), (ksw, kTs)):
                    nc.sync.dma_start(out=tsb[:, 0:H0], in_=tdr[u][:, 0:H0])
                nc.sync.dma_start(
                    out=vb_sb[:].rearrange("p (n d) -> p n d", d=128),
                    in_=vb[u].rearrange("(n p) d -> p n d", p=128),
                )
                nc.sync.dma_start(
                    out=v_sb[:].rearrange("p (n d) -> p n d", d=128),
                    in_=v8[u].rearrange("(n p) d -> p n d", p=128),
                )
                for tsb, tdr in ((qraw, qT), (kraw, kT), (qsw, qTs), (ksw, kTs)):
                    nc.sync.dma_start(out=tsb[:, H0:S], in_=tdr[u][:, H0:S])
                rq = bigpool.tile([128, S], dt.bfloat16, tag="rq")
                rk = bigpool.tile([128, S], dt.bfloat16, tag="rk")
                swm = bigpool.tile([128, S], dt.bfloat16, tag="swm")
                return dict(
                    q=qraw, k=kraw, qs=qsw, ks=ksw, v=v_sb, vb=vb_sb,
                    rq=rq, rk=rk, sw=swm,
                )

            def rotary(t, lo, hi, pool_add=False):
                """rq = q*cexp + qsw*ssig on [lo,hi); swap-mult on Pool.
                pool_add also moves the final add to Pool (for the next-unit
                rotary that runs while this unit's attention keeps DVE busy)."""
                step = 1024 if (hi - lo) % 1024 == 0 else 512
                for raw, swr, r in ((t["q"], t["qs"], t["rq"]), (t["k"], t["ks"], t["rk"])):
                    for c in range(lo // step, hi // step):
                        sl = slice(c * step, (c + 1) * step)
                        nc.gpsimd.tensor_mul(t["sw"][:, sl], swr[:, sl], ssig_sb[:, sl])
                        nc.vector.tensor_mul(r[:, sl], raw[:, sl], cexp_sb[:, sl])
                        if pool_add:
                            nc.gpsimd.tensor_add(r[:, sl], r[:, sl], t["sw"][:, sl])
                        else:
                            nc.vector.tensor_add(r[:, sl], r[:, sl], t["sw"][:, sl])

            state = {"pv": [], "tail": None}

            def flush_pv():
                if state["pv"]:
                    state["pv"].pop(0)()

            def flush_all():
                while state["pv"]:
                    flush_pv()

            def attention_qb(u, t, qb):
                groups = plan_qb(qb)
                n_groups = len(groups)
                rq, rk, v_sb = t["rq"], t["rk"], t["v"]
                qbctx = {}

                def get_psums():
                    if "outT" not in qbctx:
                        qbctx["outT"] = ps_out.tile(
                            [128, QB], dt.float32, tag="outT", name="outT"
                        )
                        qbctx["den"] = ps_den.tile(
                            [128, QB], dt.float32, tag="den", name="den"
                        )
                    return qbctx["outT"], qbctx["den"]

                bf = qb == 0  # query-block 0 runs in bf16 (see plan_qb)
                for gi, g in enumerate(groups):
                    gtiles, offs = g["tiles"], g["offs"]
                    if bf:
                        probs = ppool.tile([128, GROUP_COLS], dt.bfloat16, tag="probsb")
                    else:
                        probs = ppool.tile([128, GROUP_COLS], dt.float8e4, tag="probs")
                    sc = ps_sc.tile([128, GROUP_COLS], dt.float32, tag="sc")
                    # QK for every tile in the group
                    for tl, off in zip(gtiles, offs):
                        csl = slice(qb * QB + tl["t0"] * 128, qb * QB + tl["t0"] * 128 + tl["span"])
                        ksl = slice(tl["kj"] * 128, (tl["kj"] + 1) * 128)
                        nc.tensor.matmul(
                            sc[:, off : off + tl["span"]],
                            rk[:, ksl], rq[:, csl], start=True, stop=True,
                        )
                    # exp: one ACT instruction per contiguous act psum range
                    for lo, hi in g["act_segs"]:
                        nc.scalar.activation(
                            probs[:, lo:hi],
                            sc[:, lo:hi],
                            mybir.ActivationFunctionType.Exp,
                            scale=SCALE,
                        )
                    # stt tiles: Schraudolph exp+mask in one DVE op each
                    for tl, off in zip(gtiles, offs):
                        if tl["exp"] == "stt":
                            pname, plo, phi = tl["pat"]
                            nc.vector.scalar_tensor_tensor(
                                out=probs[:, off : off + tl["span"]].bitcast(dt.int8),
                                in0=sc[:, off : off + tl["span"]],
                                scalar=C0,
                                in1=pat_sb[pname][:, plo:phi],
                                op0=mybir.AluOpType.add,
                                op1=mybir.AluOpType.mult,
                            )
                    # post-masks for act tiles with masked chunks
                    for tl, off in zip(gtiles, offs):
                        if tl["post"] is not None:
                            c_lo, c_hi, mn = tl["post"]
                            m = mDb_sb if bf else mask_sb[mn]
                            assert not bf or mn == "mD"
                            nc.vector.tensor_mul(
                                probs[:, off + c_lo : off + c_hi],
                                probs[:, off + c_lo : off + c_hi],
                                m[:],
                            )

                    is_last = gi == n_groups - 1

                    def pv_emit(
                        u=u, qb=qb, g=g, probs=probs, gi=gi, last_group=is_last, bf=bf
                    ):
                        outT_ps, den_ps = get_psums()
                        gtiles, offs = g["tiles"], g["offs"]
                        first = gi == 0
                        n_mm = len(gtiles) - len(g["pairs"])
                        # all PV matmuls, then all den matmuls: fewer psum
                        # accumulation-target switches on the PE
                        for which in ("pv", "den"):
                            emitted = 0
                            for ti, (tl, off) in enumerate(zip(gtiles, offs)):
                                if ti - 1 in g["pairs"]:
                                    continue  # second member of a pair
                                psl = slice(
                                    tl["t0"] * 128, tl["t0"] * 128 + tl["span"]
                                )
                                is_pair = ti in g["pairs"]
                                last = last_group and emitted == n_mm - 1
                                st = first and emitted == 0
                                kj = tl["kj"]
                                tgt = outT_ps if which == "pv" else den_ps
                                if is_pair:
                                    lhs = (
                                        v_sb[:, kj * 128 : (kj + 2) * 128]
                                        if which == "pv"
                                        else ones8_sb[:, 0:256]
                                    ).rearrange("p (two d) -> p two d", two=2)
                                    nc.tensor.matmul(
                                        tgt[:, psl], lhs,
                                        probs[:, off : off + 2 * tl["span"]].rearrange(
                                            "p (two q) -> p two q", two=2
                                        ),
                                        start=st, stop=last,
                                        perf_mode=mybir.MatmulPerfMode.DoubleRow,
                                    )
                                else:
                                    if which == "pv":
                                        lhs = (
                                            t["vb"][:, kj * 128 : (kj + 1) * 128]
                                            if bf
                                            else v_sb[:, kj * 128 : (kj + 1) * 128]
                                        )
                                    else:
                                        lhs = onesb_sb[:] if bf else ones8_sb[:, 0:128]
                                    nc.tensor.matmul(
                                        tgt[:, psl], lhs,
                                        probs[:, off : off + tl["span"]],
                                        start=st, stop=last,
                                    )
                                emitted += 1

                        if last_group:
                            rden = tpool.tile([128, QB], dt.float32, tag="rden")
                            nc.vector.reciprocal_approx_fast(rden[:], den_ps[:])
                            outN = tpool.tile([128, QB], dt.bfloat16, tag="outN")
                            nc.vector.tensor_mul(outN[:], outT_ps[:], rden[:])

                            def tail(u=u, qb=qb, outN=outN):
                                out_sb = tpool.tile([128, QB], dt.bfloat16, tag="out_sb")
                                nc.sync.dma_start_transpose(
                                    out=out_sb[:].rearrange("p (n d) -> p n d", d=128),
                                    in_=outN[:],
                                )
                                nc.sync.dma_start(
                                    out=out[u].rearrange("(n p) d -> p n d", p=128)[
                                        :, qb * NQC : (qb + 1) * NQC, :
                                    ],
                                    in_=out_sb[:].rearrange("p (n d) -> p n d", d=128),
                                )

                            if state["tail"] is not None:
                                state["tail"]()
                            state["tail"] = tail

                    state["pv"].append(pv_emit)
                    if len(state["pv"]) > 2:
                        flush_pv()

            cur = load(0)
            load_consts_rest()
            # rotary runs lazily one query-block ahead of attention; the next
            # unit's first 1024 columns are pre-rotated during this unit's
            # final blocks so no engine idles at unit boundaries
            rot = [0] * per_core

            def ensure_rot(ui, t, upto):
                upto = min(upto, S)
                if upto > rot[ui]:
                    rotary(t, rot[ui], upto)
                    rot[ui] = upto

            for u in range(per_core):
                nxt = load(u + 1) if u + 1 < per_core else None
                for qb in range(nqb):
                    if u == 0 and qb == 0:
                        ensure_rot(0, cur, 512)
                    ensure_rot(u, cur, (qb + 2) * QB)
                    if nxt is not None and qb >= 4:
                        ensure_rot(u + 1, nxt, (qb - 3) * 1024)
                    attention_qb(u, cur, qb)
                cur = nxt
            flush_all()
            if state["tail"] is not None:
                state["tail"]()

    nc.compile()
    return nc


def host_prep(q, k, v, cos, sin):
    """Build per-core input maps from full inputs."""
    b, s, h, d = q.shape

    cexp = np.empty((128, s), dtype=np.float32)
    ssig = np.empty((128, s), dtype=np.float32)
    cexp[0::2, :] = cos.T
    cexp[1::2, :] = cos.T
    ssig[0::2, :] = -sin.T
    ssig[1::2, :] = sin.T
    cexp = cexp.astype(BF16)
    ssig = ssig.astype(BF16)

    ones8 = np.ones((128, 256), dtype=FP8)

    # masks in the transposed-score layout: partition p = key offset,
    # column c = query offset
    p = np.arange(128)[:, None]
    c = np.arange(128)[None, :]
    maskD = (c >= p).astype(np.float32)
    w12 = ((c - p) < PARTIAL[12]).astype(np.float32)
    w13 = ((c - p) < PARTIAL[13]).astype(np.float32)

    m0 = np.float32(M0)
    full = np.full((128, 128), m0, dtype=np.float32)
    dpat = np.concatenate([maskD * m0, full, full, full], axis=1).astype(np.float32)
    fwp = np.concatenate([full, full, w12 * m0, w13 * m0], axis=1).astype(np.float32)

    mD8 = maskD.astype(FP8)
    mW12_8 = w12.astype(FP8)
    mW13_8 = w13.astype(FP8)
    mWp8 = np.concatenate([w12, w13], axis=1).astype(FP8)

    units = [(bi, hi) for bi in range(b) for hi in range(h)]
    per = len(units) // N_CORES
    sw = np.arange(128) ^ 1  # row swap: even<->odd
    in_maps = []
    for core in range(N_CORES):
        us = units[core * per : (core + 1) * per]
        qTc = np.stack([q[bi, :, hi, :].T for bi, hi in us]).astype(BF16)
        kTc = np.stack([k[bi, :, hi, :].T for bi, hi in us]).astype(BF16)
        vc = np.stack([v[bi, :, hi, :] for bi, hi in us])
        m = {
            "qT": np.ascontiguousarray(qTc),
            "kT": np.ascontiguousarray(kTc),
            "qTs": np.ascontiguousarray(qTc[:, sw, :]),
            "kTs": np.ascontiguousarray(kTc[:, sw, :]),
            "v8": np.ascontiguousarray(vc.astype(FP8)),
            "vb": np.ascontiguousarray(vc[:, 0:512, :].astype(BF16)),
            "cexpT": cexp,
            "ssigT": ssig,
            "ones8": ones8,
            "onesb": np.ones((128, 128), dtype=BF16),
            "mDb": maskD.astype(BF16),
            "dpat": dpat,
            "fw": fwp,
            "mD": mD8,
            "mW12": mW12_8,
            "mW13": mW13_8,
            "mWp": mWp8,
        }
        in_maps.append(m)
    return in_maps, units


_NC_CACHE = {}


def kernel(q, k, v, cos, sin):
    from concourse.bass_utils import run_bass_kernel_spmd

    q = np.asarray(q, dtype=np.float32)
    k = np.asarray(k, dtype=np.float32)
    v = np.asarray(v, dtype=np.float32)
    cos = np.asarray(cos, dtype=np.float32)
    sin = np.asarray(sin, dtype=np.float32)

    if "nc" not in _NC_CACHE:
        _NC_CACHE["nc"] = build_nc()
    nc = _NC_CACHE["nc"]

    in_maps, units = host_prep(q, k, v, cos, sin)
    res = run_bass_kernel_spmd(nc, in_maps, core_ids=list(range(N_CORES)))

    b, s, h, d = q.shape
    full = np.empty((b, s, h, d), dtype=np.float32)
    per = len(units) // N_CORES
    for core in range(N_CORES):
        o = res.results[core]["out"]  # [per, s, 128] bf16
        for i, (bi, hi) in enumerate(units[core * per : (core + 1) * per]):
            full[bi, :, hi, :] = o[i].astype(np.float32)
    return full
